# revision 26
# baseline (speedup 1.0000x reference)
"""Chamfer loss Trainium2 kernel.

Problem: B=8 batches of two point clouds x:(4096,3), y:(4096,3).
  out = mean_n min_m ||x_n - y_m||^2 + mean_m min_n ||x_n - y_m||^2

Sharding: batch-parallel across 8 NeuronCores (one batch element per core).

Per-core algorithm:
  Host precomputes xp/yp feature rows (24,4096) so that
  dist[n,m] = sum_k xp[k,n]*yp[k,m]  (a single K=24 matmul).
  Device: PE emits 4096x4096 distances into PSUM (bf16 hi/mid/lo split
  operands, K=24, for fp32-grade products at full PE rate); ACT converts each
  128-row block PSUM fp32 -> SBUF bf16; DVE does rowmin via a pairwise
  tensor_tensor(min) tree (2x mode) + tensor_reduce, and a running
  elementwise colmin via tensor_tensor(min).  Tail: PE transposes of
  the colmin buffer + DVE min-reduce give per-column minima; ones-matmul does
  the cross-partition sums.  Each core returns [sum(rowmin), sum(colmin)];
  the host combines the 8 partial sums into the scalar.
"""

import sys

import numpy as np

for _p in (
    "/opt/trn_rl_repo",
    "/root/.axon_site",
    "/root/.axon_site/_ro/pypackages",
):
    if _p not in sys.path:
        sys.path.append(_p)

from concourse import bacc, mybir, tile  # noqa: E402

try:
    import ml_dtypes

    _BF16 = ml_dtypes.bfloat16
except ImportError:  # pragma: no cover
    _BF16 = np.float32

B, N, M = 8, 4096, 4096
NCORES = 8
KF = 24  # feature rows of xp/yp (bf16 hi/mid/lo split, see _prep_inputs)
NT = N // 128  # 32 row blocks
F32 = mybir.dt.float32
BF16 = mybir.dt.bfloat16


def _build_program(reps: int = 1):
    nc = bacc.Bacc(None, target_bir_lowering=False, debug=False)

    xp_d = nc.dram_tensor("xp", [KF, N], BF16, kind="ExternalInput")
    yp_d = nc.dram_tensor("yp", [KF, M], BF16, kind="ExternalInput")
    id_d = nc.dram_tensor("ident", [128, 128], BF16, kind="ExternalInput")
    out_d = nc.dram_tensor("out", [1, 2], F32, kind="ExternalOutput")

    with tile.TileContext(nc) as tc:
        with (
            tc.tile_pool(name="const", bufs=1) as cpool,
            tc.tile_pool(name="dsb", bufs=6) as dpool,
            tc.tile_pool(name="scratch", bufs=3) as spool,
        ):
            xp_sb = cpool.tile([KF, N], BF16)
            yp_sb = cpool.tile([KF, M], BF16)
            id_sb = cpool.tile([128, 128], BF16)
            nc.sync.dma_start(xp_sb[:], xp_d[:])
            nc.sync.dma_start(yp_sb[:], yp_d[:])
            nc.sync.dma_start(id_sb[:], id_d[:])

            colmin = cpool.tile([128, M], BF16)
            nc.vector.memset(colmin[:], 3.0e38)
            rowmins = cpool.tile([128, NT], F32)
            cmins = cpool.tile([128, M // 128], F32)
            sums = cpool.tile([128, 2], F32)

            xpr = xp_sb[:]
            ypr = yp_sb[:]

            with tc.tile_pool(name="psum", bufs=2, space="PSUM") as pp:
                for i in _rep_range(reps):
                    d_sb = dpool.tile([128, M], BF16)
                    for h in range(2):
                        pt = pp.tile([128, 2048], F32)
                        for jj in range(4):
                            m0 = h * 2048 + jj * 512
                            nc.tensor.matmul(
                                pt[:, jj * 512 : (jj + 1) * 512],
                                xpr[:, i * 128 : (i + 1) * 128],
                                ypr[:, m0 : m0 + 512],
                                start=True,
                                stop=True,
                            )
                        # PSUM fp32 -> SBUF bf16 (ScalarE, frees the bank)
                        nc.scalar.copy(d_sb[:, h * 2048 : (h + 1) * 2048], pt[:])
                    # rowmin of this 128-row block: pairwise TT-min tree at
                    # 2x (tensor_scalar's accum path measures 1x on HW, and
                    # tensor_tensor_reduce faults) + final 1x tensor_reduce.
                    t1 = spool.tile([128, M // 2], BF16, tag="t1")
                    nc.vector.tensor_tensor(
                        t1[:], d_sb[:, : M // 2], d_sb[:, M // 2 :],
                        op=mybir.AluOpType.min,
                    )
                    t2 = spool.tile([128, M // 4], BF16, tag="t2")
                    nc.vector.tensor_tensor(
                        t2[:], t1[:, : M // 4], t1[:, M // 4 :],
                        op=mybir.AluOpType.min,
                    )
                    t3 = spool.tile([128, M // 8], BF16, tag="t3")
                    nc.vector.tensor_tensor(
                        t3[:], t2[:, : M // 8], t2[:, M // 8 :],
                        op=mybir.AluOpType.min,
                    )
                    nc.vector.tensor_reduce(
                        rowmins[:, i : i + 1],
                        t3[:],
                        axis=mybir.AxisListType.X,
                        op=mybir.AluOpType.min,
                    )
                    # running columnwise min (DVE 2x: bf16)
                    nc.vector.tensor_tensor(
                        colmin[:], d_sb[:], colmin[:], op=mybir.AluOpType.min
                    )

            # ---- tail ----
            with tc.tile_pool(name="psum2", bufs=4, space="PSUM") as pp2:
                # 4 transposed 128x128 blocks per PSUM tile, one 3D-AP
                # min-reduce per group (8 DVE reduces instead of 32).
                for g in range(M // 512):
                    tp = pp2.tile([128, 4, 128], BF16)
                    for k in range(4):
                        b = 4 * g + k
                        nc.tensor.transpose(
                            tp[:, k, :], colmin[:, b * 128 : (b + 1) * 128],
                            id_sb[:],
                        )
                    nc.vector.tensor_reduce(
                        cmins[:, 4 * g : 4 * g + 4],
                        tp[:],
                        axis=mybir.AxisListType.X,
                        op=mybir.AluOpType.min,
                    )
                nc.vector.tensor_reduce(
                    sums[:, 0:1],
                    rowmins[:],
                    axis=mybir.AxisListType.X,
                    op=mybir.AluOpType.add,
                )
                nc.vector.tensor_reduce(
                    sums[:, 1:2],
                    cmins[:],
                    axis=mybir.AxisListType.X,
                    op=mybir.AluOpType.add,
                )
                ones_sb = cpool.tile([128, 1], F32)
                nc.vector.memset(ones_sb[:], 1.0)
                fin = pp2.tile([1, 2], F32)
                nc.tensor.matmul(fin[:], ones_sb[:], sums[:], start=True, stop=True)
                out_sb = cpool.tile([1, 2], F32)
                nc.scalar.copy(out_sb[:], fin[:])
                nc.sync.dma_start(out_d[:], out_sb[:])

    nc.compile()
    return nc


def _rep_range(reps: int):
    """NT main-loop iterations, repeated `reps` times (for HW timing)."""
    for _ in range(reps):
        yield from range(NT)


_NC_CACHE = None


def _get_nc():
    global _NC_CACHE
    if _NC_CACHE is None:
        _NC_CACHE = _build_program()
    return _NC_CACHE


def _make_runner(nc):
    """Build a cached jitted SPMD runner (mirrors bass2jax.run_bass_via_pjrt,
    but reuses one jit so repeat calls skip retracing)."""
    import jax
    from jax.experimental.shard_map import shard_map
    from jax.sharding import Mesh, PartitionSpec

    from concourse.bass2jax import (
        _bass_exec_p,
        install_neuronx_cc_hook,
        partition_id_tensor,
    )

    install_neuronx_cc_hook()
    partition_name = (
        nc.partition_id_tensor.name if nc.partition_id_tensor else None
    )
    in_names: list[str] = []
    out_names: list[str] = []
    out_avals = []
    zero_shapes = []
    for alloc in nc.m.functions[0].allocations:
        if not isinstance(alloc, mybir.MemoryLocationSet):
            continue
        name = alloc.memorylocations[0].name
        if alloc.kind == "ExternalInput":
            if name != partition_name:
                in_names.append(name)
        elif alloc.kind == "ExternalOutput":
            assert alloc.tensor_shape is not None and alloc.dtype is not None
            out_names.append(name)
            shape = tuple(alloc.tensor_shape)
            dtype = mybir.dt.np(alloc.dtype)
            out_avals.append(jax.core.ShapedArray(shape, dtype))
            zero_shapes.append((shape, dtype))
    n_params = len(in_names)
    all_in = list(in_names) + list(out_names)
    if partition_name is not None:
        all_in.append(partition_name)
    all_in = tuple(all_in)

    def _body(*args):
        operands = list(args)
        if partition_name is not None:
            operands.append(partition_id_tensor())
        outs = _bass_exec_p.bind(
            *operands,
            out_avals=tuple(out_avals),
            in_names=all_in,
            out_names=tuple(out_names),
            lowering_input_output_aliases=(),
            sim_require_finite=True,
            sim_require_nnan=True,
            nc=nc,
        )
        return tuple(outs)

    devices = jax.devices()[:NCORES]
    mesh = Mesh(np.asarray(devices), ("core",))
    nio = n_params + len(out_names)
    sharded = jax.jit(
        shard_map(
            _body,
            mesh=mesh,
            in_specs=(PartitionSpec("core"),) * nio,
            out_specs=(PartitionSpec("core"),) * len(out_names),
            check_rep=False,
        ),
        donate_argnums=tuple(range(n_params, nio)),
        keep_unused=True,
    )

    def run(in_maps):
        concat_in = [
            np.concatenate([np.asarray(m[nm]) for m in in_maps], axis=0)
            for nm in in_names
        ]
        concat_zeros = [
            np.zeros((NCORES * s[0], *s[1:]), d) for s, d in zero_shapes
        ]
        outs = sharded(*concat_in, *concat_zeros)
        return [
            {
                nm: np.asarray(outs[i]).reshape(NCORES, *out_avals[i].shape)[c]
                for i, nm in enumerate(out_names)
            }
            for c in range(NCORES)
        ]

    return run


_RUNNER_CACHE = None


def _get_runner():
    global _RUNNER_CACHE
    if _RUNNER_CACHE is None:
        _RUNNER_CACHE = _make_runner(_get_nc())
    return _RUNNER_CACHE


def _split3(v: np.ndarray):
    """Split fp64 array into three bf16 terms: v ~= h + m + l (~24 bits)."""
    h = v.astype(_BF16)
    r = v - h.astype(np.float64)
    m = r.astype(_BF16)
    r2 = r - m.astype(np.float64)
    lo = r2.astype(_BF16)
    return h, m, lo


def _prep_inputs(receptive_pc: np.ndarray, decoder_pc: np.ndarray):
    """Build per-core input maps from the full (B,N,3)/(B,M,3) arrays.

    dist[n,m] = x.x + y.y - 2 x.y is expressed as sum_k xp[k,n]*yp[k,m] in
    bf16 with hi/mid/lo splits: per coordinate the 6 product rows
    (h,h),(m,h),(h,m),(m,m),(l,h),(h,l) cover the fp32 product to ~2^-23;
    the squared norms use 3-way splits against a row of ones.
    """
    ident = np.eye(128, dtype=np.float32).astype(_BF16)
    ones = np.ones(N, dtype=_BF16)
    in_maps = []
    for b in range(B):
        x = np.asarray(receptive_pc[b], dtype=np.float64)  # (N,3)
        y = np.asarray(decoder_pc[b], dtype=np.float64)  # (M,3)
        xp = np.empty((KF, N), dtype=_BF16)
        yp = np.empty((KF, M), dtype=_BF16)
        r = 0
        for i in range(3):
            xh, xm, xl = _split3(x[:, i])
            ch, cm, cl = _split3(-2.0 * y[:, i])
            for xa, ya in ((xh, ch), (xm, ch), (xh, cm), (xm, cm), (xl, ch), (xh, cl)):
                xp[r] = xa
                yp[r] = ya
                r += 1
        x2h, x2m, x2l = _split3((x * x).sum(axis=1))
        for xa in (x2h, x2m, x2l):
            xp[r] = xa
            yp[r] = ones
            r += 1
        y2h, y2m, y2l = _split3((y * y).sum(axis=1))
        for ya in (y2h, y2m, y2l):
            xp[r] = ones
            yp[r] = ya
            r += 1
        assert r == KF
        in_maps.append({"xp": xp, "yp": yp, "ident": ident})
    return in_maps


def kernel(receptive_pc: np.ndarray, decoder_pc: np.ndarray) -> np.ndarray:
    in_maps = _prep_inputs(receptive_pc, decoder_pc)
    results = _get_runner()(in_maps)
    s1 = 0.0
    s2 = 0.0
    for b in range(B):
        o = np.asarray(results[b]["out"], dtype=np.float64).reshape(2)
        s1 += o[0]
        s2 += o[1]
    val = s1 / (B * N) + s2 / (B * M)
    return np.float32(val)


# revision 27
# speedup vs baseline: 1.5135x; 1.5135x over previous
"""Chamfer loss Trainium2 kernel.

Problem: B=8 batches of two point clouds x:(4096,3), y:(4096,3).
  out = mean_n min_m ||x_n - y_m||^2 + mean_m min_n ||x_n - y_m||^2

Sharding: batch-parallel across 8 NeuronCores (one batch element per core).

Per-core algorithm:
  Host precomputes xp/yp feature rows (24,4096) so that
  dist[n,m] = sum_k xp[k,n]*yp[k,m]  (a single K=24 matmul).
  Device: PE emits 4096x4096 distances into PSUM (bf16 hi/mid/lo split
  operands, K=24, for fp32-grade products at full PE rate); ACT converts each
  128-row block PSUM fp32 -> SBUF bf16; DVE does rowmin via a pairwise
  tensor_tensor(min) tree (2x mode) + tensor_reduce, and a running
  elementwise colmin via tensor_tensor(min).  Tail: PE transposes of
  the colmin buffer + DVE min-reduce give per-column minima; ones-matmul does
  the cross-partition sums.  Each core returns [sum(rowmin), sum(colmin)];
  the host combines the 8 partial sums into the scalar.
"""

import sys

import numpy as np

for _p in (
    "/opt/trn_rl_repo",
    "/root/.axon_site",
    "/root/.axon_site/_ro/pypackages",
):
    if _p not in sys.path:
        sys.path.append(_p)

from concourse import bacc, mybir, tile  # noqa: E402

try:
    import ml_dtypes

    _BF16 = ml_dtypes.bfloat16
except ImportError:  # pragma: no cover
    _BF16 = np.float32

B, N, M = 8, 4096, 4096
NCORES = 8
KF = 24  # feature rows of xp/yp (bf16 hi/mid/lo split, see _prep_inputs)
NT = N // 128  # 32 row blocks
F32 = mybir.dt.float32
BF16 = mybir.dt.bfloat16


def _build_program(reps: int = 1):
    nc = bacc.Bacc(None, target_bir_lowering=False, debug=False)

    xp_d = nc.dram_tensor("xp", [KF, N], BF16, kind="ExternalInput")
    yp_d = nc.dram_tensor("yp", [KF, M], BF16, kind="ExternalInput")
    id_d = nc.dram_tensor("ident", [128, 128], BF16, kind="ExternalInput")
    out_d = nc.dram_tensor("out", [1, 2], F32, kind="ExternalOutput")

    with tile.TileContext(nc) as tc:
        with (
            tc.tile_pool(name="const", bufs=1) as cpool,
            tc.tile_pool(name="dsb", bufs=8) as dpool,
            tc.tile_pool(name="scratch", bufs=4) as spool,
        ):
            xp_sb = cpool.tile([KF, N], BF16)
            yp_sb = cpool.tile([KF, M], BF16)
            id_sb = cpool.tile([128, 128], BF16)
            # first chunks unblock matmul i=0 early; bulk follows
            nc.sync.dma_start(xp_sb[:, :128], xp_d[:, :128])
            nc.sync.dma_start(yp_sb[:, :2048], yp_d[:, :2048])
            nc.sync.dma_start(xp_sb[:, 128:], xp_d[:, 128:])
            nc.sync.dma_start(yp_sb[:, 2048:], yp_d[:, 2048:])
            nc.sync.dma_start(id_sb[:], id_d[:])

            colmin = cpool.tile([128, M], BF16)
            nc.vector.memset(colmin[:], 3.0e38)
            rowmins = cpool.tile([128, NT], F32)
            cmins = cpool.tile([128, M // 128], F32)
            sums = cpool.tile([128, 2], F32)

            xpr = xp_sb[:]
            ypr = yp_sb[:]

            with tc.tile_pool(name="psum", bufs=2, space="PSUM") as pp:
                for i in _rep_range(reps):
                    d_sb = dpool.tile([128, M], BF16)
                    for h in range(2):
                        pt = pp.tile([128, 2048], F32)
                        for jj in range(4):
                            m0 = h * 2048 + jj * 512
                            nc.tensor.matmul(
                                pt[:, jj * 512 : (jj + 1) * 512],
                                xpr[:, i * 128 : (i + 1) * 128],
                                ypr[:, m0 : m0 + 512],
                                start=True,
                                stop=True,
                            )
                        # PSUM fp32 -> SBUF bf16 (ScalarE, frees the bank)
                        nc.scalar.copy(d_sb[:, h * 2048 : (h + 1) * 2048], pt[:])
                    # running columnwise min first: after the last block's
                    # colmin the tail's PE transposes overlap the final tree.
                    nc.vector.tensor_tensor(
                        colmin[:], d_sb[:], colmin[:], op=mybir.AluOpType.min
                    )
                    # rowmin of this 128-row block: pairwise TT-min tree at
                    # 2x (tensor_scalar's accum path measures 1x on HW, and
                    # tensor_tensor_reduce faults) + final 1x tensor_reduce.
                    t1 = spool.tile([128, M // 2], BF16, tag="t1")
                    nc.vector.tensor_tensor(
                        t1[:], d_sb[:, : M // 2], d_sb[:, M // 2 :],
                        op=mybir.AluOpType.min,
                    )
                    t2 = spool.tile([128, M // 4], BF16, tag="t2")
                    nc.vector.tensor_tensor(
                        t2[:], t1[:, : M // 4], t1[:, M // 4 :],
                        op=mybir.AluOpType.min,
                    )
                    t3 = spool.tile([128, M // 8], BF16, tag="t3")
                    nc.vector.tensor_tensor(
                        t3[:], t2[:, : M // 8], t2[:, M // 8 :],
                        op=mybir.AluOpType.min,
                    )
                    nc.vector.tensor_reduce(
                        rowmins[:, i : i + 1],
                        t3[:],
                        axis=mybir.AxisListType.X,
                        op=mybir.AluOpType.min,
                    )

            # ---- tail ----
            with tc.tile_pool(name="psum2", bufs=4, space="PSUM") as pp2:
                # 4 transposed 128x128 blocks per PSUM tile, one 3D-AP
                # min-reduce per group (8 DVE reduces instead of 32).
                for g in range(M // 512):
                    tp = pp2.tile([128, 4, 128], BF16)
                    for k in range(4):
                        b = 4 * g + k
                        nc.tensor.transpose(
                            tp[:, k, :], colmin[:, b * 128 : (b + 1) * 128],
                            id_sb[:],
                        )
                    nc.vector.tensor_reduce(
                        cmins[:, 4 * g : 4 * g + 4],
                        tp[:],
                        axis=mybir.AxisListType.X,
                        op=mybir.AluOpType.min,
                    )
                nc.vector.tensor_reduce(
                    sums[:, 0:1],
                    rowmins[:],
                    axis=mybir.AxisListType.X,
                    op=mybir.AluOpType.add,
                )
                nc.vector.tensor_reduce(
                    sums[:, 1:2],
                    cmins[:],
                    axis=mybir.AxisListType.X,
                    op=mybir.AluOpType.add,
                )
                ones_sb = cpool.tile([128, 1], F32)
                nc.vector.memset(ones_sb[:], 1.0)
                fin = pp2.tile([1, 2], F32)
                nc.tensor.matmul(fin[:], ones_sb[:], sums[:], start=True, stop=True)
                out_sb = cpool.tile([1, 2], F32)
                nc.scalar.copy(out_sb[:], fin[:])
                nc.sync.dma_start(out_d[:], out_sb[:])

    nc.compile()
    return nc


def _rep_range(reps: int):
    """NT main-loop iterations, repeated `reps` times (for HW timing)."""
    for _ in range(reps):
        yield from range(NT)


_NC_CACHE = None


def _get_nc():
    global _NC_CACHE
    if _NC_CACHE is None:
        _NC_CACHE = _build_program()
    return _NC_CACHE


def _make_runner(nc):
    """Build a cached jitted SPMD runner (mirrors bass2jax.run_bass_via_pjrt,
    but reuses one jit so repeat calls skip retracing)."""
    import jax
    from jax.experimental.shard_map import shard_map
    from jax.sharding import Mesh, PartitionSpec

    from concourse.bass2jax import (
        _bass_exec_p,
        install_neuronx_cc_hook,
        partition_id_tensor,
    )

    install_neuronx_cc_hook()
    partition_name = (
        nc.partition_id_tensor.name if nc.partition_id_tensor else None
    )
    in_names: list[str] = []
    out_names: list[str] = []
    out_avals = []
    zero_shapes = []
    for alloc in nc.m.functions[0].allocations:
        if not isinstance(alloc, mybir.MemoryLocationSet):
            continue
        name = alloc.memorylocations[0].name
        if alloc.kind == "ExternalInput":
            if name != partition_name:
                in_names.append(name)
        elif alloc.kind == "ExternalOutput":
            assert alloc.tensor_shape is not None and alloc.dtype is not None
            out_names.append(name)
            shape = tuple(alloc.tensor_shape)
            dtype = mybir.dt.np(alloc.dtype)
            out_avals.append(jax.core.ShapedArray(shape, dtype))
            zero_shapes.append((shape, dtype))
    n_params = len(in_names)
    all_in = list(in_names) + list(out_names)
    if partition_name is not None:
        all_in.append(partition_name)
    all_in = tuple(all_in)

    def _body(*args):
        operands = list(args)
        if partition_name is not None:
            operands.append(partition_id_tensor())
        outs = _bass_exec_p.bind(
            *operands,
            out_avals=tuple(out_avals),
            in_names=all_in,
            out_names=tuple(out_names),
            lowering_input_output_aliases=(),
            sim_require_finite=True,
            sim_require_nnan=True,
            nc=nc,
        )
        return tuple(outs)

    devices = jax.devices()[:NCORES]
    mesh = Mesh(np.asarray(devices), ("core",))
    nio = n_params + len(out_names)
    sharded = jax.jit(
        shard_map(
            _body,
            mesh=mesh,
            in_specs=(PartitionSpec("core"),) * nio,
            out_specs=(PartitionSpec("core"),) * len(out_names),
            check_rep=False,
        ),
        donate_argnums=tuple(range(n_params, nio)),
        keep_unused=True,
    )

    def run(in_maps):
        concat_in = [
            np.concatenate([np.asarray(m[nm]) for m in in_maps], axis=0)
            for nm in in_names
        ]
        concat_zeros = [
            np.zeros((NCORES * s[0], *s[1:]), d) for s, d in zero_shapes
        ]
        outs = sharded(*concat_in, *concat_zeros)
        return [
            {
                nm: np.asarray(outs[i]).reshape(NCORES, *out_avals[i].shape)[c]
                for i, nm in enumerate(out_names)
            }
            for c in range(NCORES)
        ]

    return run


_RUNNER_CACHE = None


def _get_runner():
    global _RUNNER_CACHE
    if _RUNNER_CACHE is None:
        _RUNNER_CACHE = _make_runner(_get_nc())
    return _RUNNER_CACHE


def _split3(v: np.ndarray):
    """Split fp64 array into three bf16 terms: v ~= h + m + l (~24 bits)."""
    h = v.astype(_BF16)
    r = v - h.astype(np.float64)
    m = r.astype(_BF16)
    r2 = r - m.astype(np.float64)
    lo = r2.astype(_BF16)
    return h, m, lo


def _prep_inputs(receptive_pc: np.ndarray, decoder_pc: np.ndarray):
    """Build per-core input maps from the full (B,N,3)/(B,M,3) arrays.

    dist[n,m] = x.x + y.y - 2 x.y is expressed as sum_k xp[k,n]*yp[k,m] in
    bf16 with hi/mid/lo splits: per coordinate the 6 product rows
    (h,h),(m,h),(h,m),(m,m),(l,h),(h,l) cover the fp32 product to ~2^-23;
    the squared norms use 3-way splits against a row of ones.
    """
    ident = np.eye(128, dtype=np.float32).astype(_BF16)
    ones = np.ones(N, dtype=_BF16)
    in_maps = []
    for b in range(B):
        x = np.asarray(receptive_pc[b], dtype=np.float64)  # (N,3)
        y = np.asarray(decoder_pc[b], dtype=np.float64)  # (M,3)
        xp = np.empty((KF, N), dtype=_BF16)
        yp = np.empty((KF, M), dtype=_BF16)
        r = 0
        for i in range(3):
            xh, xm, xl = _split3(x[:, i])
            ch, cm, cl = _split3(-2.0 * y[:, i])
            for xa, ya in ((xh, ch), (xm, ch), (xh, cm), (xm, cm), (xl, ch), (xh, cl)):
                xp[r] = xa
                yp[r] = ya
                r += 1
        x2h, x2m, x2l = _split3((x * x).sum(axis=1))
        for xa in (x2h, x2m, x2l):
            xp[r] = xa
            yp[r] = ones
            r += 1
        y2h, y2m, y2l = _split3((y * y).sum(axis=1))
        for ya in (y2h, y2m, y2l):
            xp[r] = ones
            yp[r] = ya
            r += 1
        assert r == KF
        in_maps.append({"xp": xp, "yp": yp, "ident": ident})
    return in_maps


def kernel(receptive_pc: np.ndarray, decoder_pc: np.ndarray) -> np.ndarray:
    in_maps = _prep_inputs(receptive_pc, decoder_pc)
    results = _get_runner()(in_maps)
    s1 = 0.0
    s2 = 0.0
    for b in range(B):
        o = np.asarray(results[b]["out"], dtype=np.float64).reshape(2)
        s1 += o[0]
        s2 += o[1]
    val = s1 / (B * N) + s2 / (B * M)
    return np.float32(val)
